# revision 1
# baseline (speedup 1.0000x reference)
"""DynamicLoRAAttention Trainium2 kernel (8 NeuronCores, SPMD).

Sharding: batch b = core//4 selects the 4-core group; within a group each
core owns 4 heads (4*rank..4*rank+3) for QKV projection + attention over
the full sequence, then a per-head-position AllGather reshards attn_out so
each core O-projects only its two frames (rank, 7-rank).  LayerNorm and the
LoRA low/gate factors are computed on every core (duplicated) to keep the
program SPMD-uniform.  The frame-block-causal mask is hardcoded as loop
ranges (frames of 256 tokens, causal over 8 frames).
"""
import numpy as np
import ml_dtypes

B, T, DIM = 2, 2048, 1024
HEADS, DH = 16, 64
INNER = HEADS * DH
R = 8
NP, NF = 256, 8
EPS = 1e-5
LORA_SCALE = 0.25
ATT_SCALE = DH ** -0.5
N_CORES = 8
BF16 = ml_dtypes.bfloat16

# inner permutation induced by the per-head-pair AllGather:
# received block (pp, i) holds heads {4*i+2*pp, 4*i+2*pp+1} (128 rows).
PERM = np.array(
    [(4 * i + 2 * pp + l) * DH + d
     for pp in range(2) for i in range(4) for l in range(2) for d in range(DH)],
    dtype=np.int64,
)


def _prep(inputs):
    """Host-side sharding/folding. Returns (in_maps, meta)."""
    f32 = np.float32
    get = lambda k: np.asarray(inputs[k], dtype=f32)
    x, m = get("x"), get("m_tok")
    g, b_ = get("norm_g"), get("norm_b")
    gm, bm = get("mnorm_g"), get("mnorm_b")
    assert np.all(b_ == 0) and np.all(bm == 0), "nonzero LN bias not supported"

    Wq, Aq, Bq, Gq = get("Wq"), get("Aq"), get("Bq"), get("Gq")
    Wk, Ak, Bk, Gk = get("Wk"), get("Ak"), get("Bk"), get("Gk")
    Wv, Av, Bv, Gv = get("Wv"), get("Av"), get("Bv"), get("Gv")
    Wo, Ao, Bo, Go = get("Wo"), get("Ao"), get("Bo"), get("Go")

    bf = lambda a: np.ascontiguousarray(a, dtype=f32).astype(BF16)

    # g folded into W/A/G; attention scale folded into q-path weights.
    Wq_g = Wq * g[None, :] * ATT_SCALE
    Wk_g = Wk * g[None, :]
    Wv_g = Wv * g[None, :]
    AallT = bf(np.concatenate([Aq * g, Ak * g, Av * g], axis=0).T)   # [1024,24]
    GallT = bf(np.concatenate([Gq * gm, Gk * gm, Gv * gm, Go * gm], axis=0).T)  # [1024,32]
    WoTp = bf(Wo.T[PERM, :])            # [1024,1024] rows permuted
    AoTp = bf(Ao.T[PERM, :])            # [1024,8]
    BoT = bf((Bo * LORA_SCALE).T)       # [8,1024]
    id128 = np.eye(128, dtype=BF16)

    shared = {
        "aallt": AallT, "gallt": GallT, "wot": WoTp, "aot": AoTp,
        "bot": BoT, "id128": id128,
    }
    in_maps = []
    for c in range(N_CORES):
        b = c // 4
        r = c % 4
        rows = slice(4 * r * DH, (4 * r + 4) * DH)   # this core's 4 heads
        im = dict(shared)
        im["x"] = np.ascontiguousarray(x[b])
        im["m"] = np.ascontiguousarray(m[b])
        im["wqt"] = bf(Wq_g[rows].T)                 # [1024,256]
        im["wkt"] = bf(Wk_g[rows].T)
        im["wvt"] = bf(Wv_g[rows].T)
        im["bqt"] = bf((Bq[rows] * LORA_SCALE * ATT_SCALE).T)   # [8,256]
        im["bkt"] = bf((Bk[rows] * LORA_SCALE).T)
        im["bvt"] = bf((Bv[rows] * LORA_SCALE).T)
        im["toff"] = np.array([[r * NP, (7 - r) * NP]], dtype=np.int32)
        in_maps.append(im)
    return in_maps


def _assemble(results):
    y = np.zeros((B, T, DIM), dtype=np.float32)
    for c in range(N_CORES):
        b, r = c // 4, c % 4
        yl = results[c]["y"]
        y[b, r * NP:(r + 1) * NP] = yl[0:NP]
        y[b, (7 - r) * NP:(8 - r) * NP] = yl[NP:2 * NP]
    return y


def build_nc(stage=4, reps=1):
    import concourse.bass as bass
    import concourse.tile as tile
    from concourse import bacc, mybir

    FP32 = mybir.dt.float32
    BF = mybir.dt.bfloat16
    AF = mybir.ActivationFunctionType

    nc = bacc.Bacc("TRN2", target_bir_lowering=False, debug=False,
                   num_devices=N_CORES)
    dt_ = nc.dram_tensor
    x_d = dt_("x", [T, DIM], FP32, kind="ExternalInput").ap()
    m_d = dt_("m", [T, DIM], FP32, kind="ExternalInput").ap()
    wqt_d = dt_("wqt", [DIM, 256], BF, kind="ExternalInput").ap()
    wkt_d = dt_("wkt", [DIM, 256], BF, kind="ExternalInput").ap()
    wvt_d = dt_("wvt", [DIM, 256], BF, kind="ExternalInput").ap()
    wot_d = dt_("wot", [INNER, DIM], BF, kind="ExternalInput").ap()
    aallt_d = dt_("aallt", [DIM, 24], BF, kind="ExternalInput").ap()
    gallt_d = dt_("gallt", [DIM, 32], BF, kind="ExternalInput").ap()
    aot_d = dt_("aot", [INNER, R], BF, kind="ExternalInput").ap()
    bqt_d = dt_("bqt", [R, 256], BF, kind="ExternalInput").ap()
    bkt_d = dt_("bkt", [R, 256], BF, kind="ExternalInput").ap()
    bvt_d = dt_("bvt", [R, 256], BF, kind="ExternalInput").ap()
    bot_d = dt_("bot", [R, DIM], BF, kind="ExternalInput").ap()
    id_d = dt_("id128", [128, 128], BF, kind="ExternalInput").ap()
    toff_d = dt_("toff", [1, 2], mybir.dt.int32, kind="ExternalInput").ap()
    y_d = dt_("y", [512, DIM], FP32, kind="ExternalOutput").ap()

    NT = T // 128          # 16 token tiles
    NDC = DIM // 128       # 8 contraction chunks

    with tile.TileContext(nc) as tc:
        with tc.tile_pool(name="persist", bufs=1) as P, \
             tc.tile_pool(name="dram", bufs=1, space="DRAM") as DR:
            # ---- persistent SBUF tensors ----
            xsT = P.tile([128, NDC, T], BF)          # xs^T, d-chunk major
            msT = P.tile([128, NDC, T], BF)
            qT = P.tile([128, 2, T], BF)             # q^T (4 heads = 256 rows)
            kT = P.tile([128, 2, T], BF)
            v_sb = P.tile([128, NT, 4, DH + 1], BF)  # v + ones col, per kv chunk
            gateT = P.tile([32, T], FP32)
            lowgT = P.tile([24, T], BF)
            payload = [P.tile([128, T], BF, name=f"payload{pp}")
                       for pp in range(2)]
            oT = [P.tile([128, 4, 2, NP], BF, name=f"oT{pp}") for pp in range(2)]
            lowg_k8 = P.tile([R, T], BF)
            lowg_v8 = P.tile([R, T], BF)
            gateo = P.tile([32, 512], FP32)
            gateo8 = P.tile([R, 512], FP32)
            lowgo8 = P.tile([R, 512], BF)
            # weights
            wqt = P.tile([128, NDC, 256], BF)
            wkt = P.tile([128, NDC, 256], BF)
            wvt = P.tile([128, NDC, 256], BF)
            wot = P.tile([128, NDC, DIM], BF)
            aallt = P.tile([128, NDC, 24], BF)
            gallt = P.tile([128, NDC, 32], BF)
            aot = P.tile([128, NDC, R], BF)
            bqt = P.tile([R, 256], BF)
            bkt = P.tile([R, 256], BF)
            bvt = P.tile([R, 256], BF)
            bot = P.tile([R, DIM], BF)
            id128 = P.tile([128, 128], BF)
            toff = P.tile([1, 2], mybir.dt.int32)

            ag_in_r = [[DR.tile([128, T], BF, name=f"agi{pp}_{rr}")
                        for pp in range(2)] for rr in range(reps)]
            ag_out_r = [[DR.tile([4, 128, T], BF, name=f"ago{pp}_{rr}")
                         for pp in range(2)] for rr in range(reps)]

            for _rep in range(reps):
                ag_in, ag_out = ag_in_r[_rep], ag_out_r[_rep]
                dma = nc.sync.dma_start
                dma(wqt[:], wqt_d.rearrange("(c p) n -> p c n", p=128))
                dma(wkt[:], wkt_d.rearrange("(c p) n -> p c n", p=128))
                dma(wvt[:], wvt_d.rearrange("(c p) n -> p c n", p=128))
                dma(wot[:], wot_d.rearrange("(c p) n -> p c n", p=128))
                dma(aallt[:], aallt_d.rearrange("(c p) n -> p c n", p=128))
                dma(gallt[:], gallt_d.rearrange("(c p) n -> p c n", p=128))
                dma(aot[:], aot_d.rearrange("(c p) n -> p c n", p=128))
                dma(bqt[:], bqt_d)
                dma(bkt[:], bkt_d)
                dma(bvt[:], bvt_d)
                dma(bot[:], bot_d)
                dma(id128[:], id_d)
                dma(toff[:], toff_d)
                nc.vector.memset(v_sb[:, :, :, DH], 1.0)

                # AllGather bounce buffers

                # ================= Phase A: LN + transpose + factors + QKV =====
                with tc.tile_pool(name="lnw", bufs=6) as LW, \
                     tc.tile_pool(name="lns", bufs=8) as LS, \
                     tc.tile_pool(name="pstr", bufs=8, space="PSUM") as PTR:
                    for src_d, dstT in ((x_d, xsT), (m_d, msT)):
                        for tt in range(NT):
                            xt = LW.tile([128, DIM], FP32, tag="xt")
                            nc.sync.dma_start(xt[:], src_d[tt * 128:(tt + 1) * 128, :])
                            stats = LS.tile([128, 2, 6], FP32, tag="st")
                            nc.vector.bn_stats(stats[:, 0, :], xt[:, 0:512])
                            nc.vector.bn_stats(stats[:, 1, :], xt[:, 512:1024])
                            mv = LS.tile([128, 2], FP32, tag="mv")
                            nc.vector.bn_aggr(mv[:], stats[:])
                            veps = LS.tile([128, 1], FP32, tag="ve")
                            nc.vector.tensor_scalar_add(veps[:], mv[:, 1:2], EPS)
                            sd = LS.tile([128, 1], FP32, tag="sd")
                            nc.scalar.activation(sd[:], veps[:], AF.Sqrt)
                            rstd = LS.tile([128, 1], FP32, tag="rs")
                            nc.vector.reciprocal(rstd[:], sd[:])
                            nmr = LS.tile([128, 1], FP32, tag="nm")
                            nc.vector.tensor_scalar(
                                nmr[:], mv[:, 0:1], rstd[:], -1.0,
                                mybir.AluOpType.mult, mybir.AluOpType.mult)
                            xs = LW.tile([128, DIM], BF, tag="xs")
                            nc.scalar.activation(xs[:], xt[:], AF.Identity,
                                                 bias=nmr[:], scale=rstd[:])
                            for dg in range(2):
                                tp = PTR.tile([128, 4, 128], BF, tag="tr")
                                for k in range(4):
                                    dc = dg * 4 + k
                                    nc.tensor.transpose(
                                        tp[:, k, :],
                                        xs[:, dc * 128:(dc + 1) * 128], id128[:])
                                dst = dstT[:, dg * 4:(dg + 1) * 4,
                                           tt * 128:(tt + 1) * 128]
                                if dg == 0:
                                    nc.vector.tensor_copy(dst, tp[:])
                                else:
                                    nc.scalar.copy(dst, tp[:])

                if stage >= 1:
                    # low/gate factors, full T
                    with tc.tile_pool(name="pslg", bufs=2, space="PSUM") as PLG, \
                         tc.tile_pool(name="lgs", bufs=2) as LGS:
                        for nt in range(4):
                            sl = slice(nt * 512, (nt + 1) * 512)
                            gp = PLG.tile([32, 512], FP32, tag="gp")
                            for dc in range(NDC):
                                nc.tensor.matmul(gp[:], gallt[:, dc, :], msT[:, dc, sl],
                                                 start=(dc == 0), stop=(dc == NDC - 1))
                            nc.vector.tensor_copy(gateT[:, sl], gp[:])
                            lp = PLG.tile([24, 512], FP32, tag="lp")
                            for dc in range(NDC):
                                nc.tensor.matmul(lp[:], aallt[:, dc, :], xsT[:, dc, sl],
                                                 start=(dc == 0), stop=(dc == NDC - 1))
                            nc.vector.tensor_mul(lowgT[:, sl], lp[:], gateT[0:24, sl])
                            nc.sync.dma_start(lowg_k8[:, sl], lowgT[8:16, sl])
                            nc.sync.dma_start(lowg_v8[:, sl], lowgT[16:24, sl])

                    # Q/K projections (orientation a: [o, t])
                    with tc.tile_pool(name="psqk", bufs=4, space="PSUM") as PQK:
                        for wt, bt, lsrc, dstT in ((wqt, bqt, None, qT),
                                                   (wkt, bkt, lowg_k8, kT)):
                            for ot in range(2):
                                for nt in range(4):
                                    sl = slice(nt * 512, (nt + 1) * 512)
                                    pp = PQK.tile([128, 512], FP32, tag="qk")
                                    for dc in range(NDC):
                                        nc.tensor.matmul(
                                            pp[:], wt[:, dc, ot * 128:(ot + 1) * 128],
                                            xsT[:, dc, sl], start=(dc == 0), stop=False)
                                    lrhs = (lowgT[0:8, sl] if lsrc is None
                                            else lsrc[:, sl])
                                    nc.tensor.matmul(
                                        pp[:], bt[:, ot * 128:(ot + 1) * 128],
                                        lrhs, start=False, stop=True)
                                    if nt % 2 == 0:
                                        nc.vector.tensor_copy(dstT[:, ot, sl], pp[:])
                                    else:
                                        nc.scalar.copy(dstT[:, ot, sl], pp[:])

                    # V projection (orientation b: [t, o])
                    with tc.tile_pool(name="psv", bufs=4, space="PSUM") as PV:
                        for tt in range(NT):
                            tsl = slice(tt * 128, (tt + 1) * 128)
                            pv = PV.tile([128, 4, DH], FP32, tag="v")
                            pvf = pv[:].rearrange("p a b -> p (a b)")
                            for dc in range(NDC):
                                nc.tensor.matmul(pvf, xsT[:, dc, tsl], wvt[:, dc, :],
                                                 start=(dc == 0), stop=False)
                            nc.tensor.matmul(pvf, lowg_v8[:, tsl], bvt[:],
                                             start=False, stop=True)
                            if tt % 2 == 0:
                                nc.vector.tensor_copy(v_sb[:, tt, :, 0:DH], pv[:])
                            else:
                                nc.scalar.copy(v_sb[:, tt, :, 0:DH], pv[:])

                # ================= Phase B: attention =========================
                if stage >= 2:
                    with tc.tile_pool(name="pss", bufs=2, space="PSUM") as PS, \
                         tc.tile_pool(name="psav", bufs=1, space="PSUM") as PAV, \
                         tc.tile_pool(name="att", bufs=8) as ATS:
                        for p in range(4):
                            pp, pi = p // 2, p % 2
                            prow = slice(pi * 64, pi * 64 + 64)
                            pot = p // 2
                            for j in range(NF // 2):
                                q2sl = slice(2 * j * NP, (2 * j + 2) * NP)
                                qs_hi = slice((2 * j + 1) * NP,
                                              (2 * j + 2) * NP)
                                nsh = 4 * j + 2
                                nkc = 4 * j + 4
                                avs = [PAV.tile([128, DH + 1], FP32,
                                                tag=f"av{i}", name=f"av{i}")
                                       for i in range(4)]
                                for kcp in range(nkc // 2):
                                    sp = PS.tile([128, 2, 512], FP32, tag="s")
                                    solo = 2 * kcp >= nsh
                                    for k2 in range(2):
                                        kc = 2 * kcp + k2
                                        if kc < nsh:
                                            nc.tensor.matmul(
                                                sp[:, k2, 0:512],
                                                kT[prow, pot,
                                                   kc * 128:(kc + 1) * 128],
                                                qT[prow, pot, q2sl],
                                                start=True, stop=True)
                                        else:
                                            nc.tensor.matmul(
                                                sp[:, k2, 256:512],
                                                kT[prow, pot,
                                                   kc * 128:(kc + 1) * 128],
                                                qT[prow, pot, qs_hi],
                                                start=True, stop=True)
                                    if solo:
                                        ex = ATS.tile([128, 2, NP], BF,
                                                      tag="exs")
                                        nc.scalar.activation(
                                            ex[:], sp[:, :, 256:512], AF.Exp)
                                        exoff = -256
                                    else:
                                        ex = ATS.tile([128, 2, 512], BF,
                                                      tag="ex")
                                        nc.scalar.activation(
                                            ex[:], sp[:], AF.Exp)
                                        exoff = 0
                                    for k2 in range(2):
                                        kc = 2 * kcp + k2
                                        frs = (0, 1) if kc < nsh else (1,)
                                        for fr in frs:
                                            for sub in range(2):
                                                cst = (fr * 256 + sub * 128
                                                       + exoff)
                                                nc.tensor.matmul(
                                                    avs[2 * fr + sub][:],
                                                    ex[:, k2,
                                                       cst:cst + 128],
                                                    v_sb[:, kc, p, :],
                                                    start=(kc == 0),
                                                    stop=(kc == (
                                                        nsh - 1 if fr == 0
                                                        else nkc - 1)))
                                for fr in range(2):
                                    for sub in range(2):
                                        av = avs[2 * fr + sub]
                                        rc = ATS.tile([128, 1], FP32,
                                                      tag="rc")
                                        nc.vector.reciprocal(
                                            rc[:], av[:, DH:DH + 1])
                                        qt = 2 * (2 * j + fr) + sub
                                        nc.vector.tensor_scalar_mul(
                                            payload[pp][:,
                                                        qt * 128 + pi * 64:
                                                        qt * 128 + pi * 64
                                                        + 64],
                                            av[:, 0:DH], rc[:])
                            if stage >= 3 and p % 2 == 1:
                                nc.sync.dma_start(ag_in[pp][:],
                                                  payload[pp][:])
                                nc.gpsimd.collective_compute(
                                    "AllGather", mybir.AluOpType.bypass,
                                    replica_groups=[[0, 1, 2, 3],
                                                    [4, 5, 6, 7]],
                                    ins=[ag_in[pp][:].opt()],
                                    outs=[ag_out[pp][:].opt()])


            # ================= Phase C: O projection ======================
                if stage >= 4:
                    offs = [nc.values_load(toff[0:1, i:i + 1], min_val=0,
                                           max_val=T - NP,
                                           skip_runtime_bounds_check=True)
                            for i in range(2)]
                    oraw = [P.tile([128, 4, 2, NP], BF, name=f"oraw{pp}")
                            for pp in range(2)]
                    for pp in range(2):
                        for fr in range(2):
                            for i in range(4):
                                nc.sync.dma_start(
                                    oraw[pp][:, i, fr, :],
                                    ag_out[pp][i, :, bass.ds(offs[fr], NP)])
                    with tc.tile_pool(name="pctr", bufs=4,
                                      space="PSUM") as PCT:
                        for pp in range(2):
                            for i in range(4):
                                for fr in range(2):
                                    for qs in range(2):
                                        ctp = PCT.tile([128, 128], BF,
                                                       tag="ct")
                                        nc.tensor.transpose(
                                            ctp[:],
                                            oraw[pp][:, i, fr,
                                                     qs * 128:
                                                     (qs + 1) * 128],
                                            id128[:])
                                        dst = oT[pp][:, i, fr,
                                                     qs * 128:
                                                     (qs + 1) * 128]
                                        if (i + fr) % 2 == 0:
                                            nc.vector.tensor_copy(dst,
                                                                  ctp[:])
                                        else:
                                            nc.scalar.copy(dst, ctp[:])
                    for fr in range(2):
                        nc.vector.tensor_copy(gateo[:, fr * NP:(fr + 1) * NP],
                                              gateT[:, bass.ds(offs[fr], NP)])
                    nc.sync.dma_start(gateo8[:], gateo[24:32, :])

                    with tc.tile_pool(name="pslo", bufs=1, space="PSUM") as PLO, \
                         tc.tile_pool(name="pso", bufs=1, space="PSUM") as PO, \
                         tc.tile_pool(name="osb", bufs=3) as OS:
                        lop = PLO.tile([R, 512], FP32)
                        for pp in range(2):
                            for i in range(4):
                                nc.tensor.matmul(
                                    lop[:], aot[:, 4 * pp + i, :],
                                    oT[pp][:, i, :, :].rearrange("p a b -> p (a b)"),
                                    start=(pp == 0 and i == 0),
                                    stop=(pp == 1 and i == 3))
                        nc.vector.tensor_mul(lowgo8[:], lop[:], gateo8[:])
                        for half in range(2):
                            hsl = slice(half * 512, (half + 1) * 512)
                            ops = [PO.tile([128, 512], FP32, tag=f"o{t}",
                                           name=f"o{t}") for t in range(4)]
                            for pp in range(2):
                                for i in range(4):
                                    for tt4 in range(4):
                                        fr, ch = tt4 // 2, tt4 % 2
                                        nc.tensor.matmul(
                                            ops[tt4][:],
                                            oT[pp][:, i, fr,
                                                   ch * 128:(ch + 1) * 128],
                                            wot[:, 4 * pp + i, hsl],
                                            start=(pp == 0 and i == 0),
                                            stop=False)
                            for tt4 in range(4):
                                nc.tensor.matmul(
                                    ops[tt4][:],
                                    lowgo8[:, tt4 * 128:(tt4 + 1) * 128],
                                    bot[:, hsl], start=False, stop=True)
                            for tt4 in range(4):
                                ys = OS.tile([128, 512], FP32, tag="y")
                                nc.scalar.copy(ys[:], ops[tt4][:])
                                nc.sync.dma_start(
                                    y_d[tt4 * 128:(tt4 + 1) * 128, hsl], ys[:])

    nc.compile()
    return nc


_NC_CACHE = None


def kernel(**inputs):
    global _NC_CACHE
    from concourse import bass_utils
    in_maps = _prep(inputs)
    if _NC_CACHE is None:
        _NC_CACHE = build_nc()
    res = bass_utils.run_bass_kernel_spmd(
        _NC_CACHE, in_maps, core_ids=list(range(N_CORES)))
    return _assemble(res.results)


def benchmark(inputs, iters=10):
    """Steady-state device execution time (ns): device-resident inputs,
    repeated dispatch of the compiled NEFF, min over iters."""
    global _NC_CACHE
    import time
    import jax
    from jax.sharding import Mesh, PartitionSpec, NamedSharding
    from jax.experimental.shard_map import shard_map
    from concourse import bass2jax, mybir

    if _NC_CACHE is None:
        _NC_CACHE = build_nc()
    nc = _NC_CACHE
    in_maps = _prep(inputs)
    bass2jax.install_neuronx_cc_hook()

    partition_name = (nc.partition_id_tensor.name
                      if nc.partition_id_tensor else None)
    in_names, out_names, out_avals, zero_outs = [], [], [], []
    for alloc in nc.m.functions[0].allocations:
        if not isinstance(alloc, mybir.MemoryLocationSet):
            continue
        name = alloc.memorylocations[0].name
        if alloc.kind == "ExternalInput":
            if name != partition_name:
                in_names.append(name)
        elif alloc.kind == "ExternalOutput":
            shape = tuple(alloc.tensor_shape)
            dtype = mybir.dt.np(alloc.dtype)
            out_names.append(name)
            out_avals.append(jax.core.ShapedArray(shape, dtype))
            zero_outs.append(np.zeros(shape, dtype))
    n_params = len(in_names)
    n_outs = len(out_avals)
    in_names = in_names + out_names
    if partition_name is not None:
        in_names.append(partition_name)
    donate = tuple(range(n_params, n_params + n_outs))

    def _body(*args):
        operands = list(args)
        if partition_name is not None:
            operands.append(bass2jax.partition_id_tensor())
        outs = bass2jax._bass_exec_p.bind(
            *operands,
            out_avals=tuple(out_avals),
            in_names=tuple(in_names),
            out_names=tuple(out_names),
            lowering_input_output_aliases=(),
            sim_require_finite=True,
            sim_require_nnan=True,
            nc=nc,
        )
        return tuple(outs)

    devices = jax.devices()[:N_CORES]
    mesh = Mesh(np.asarray(devices), ("core",))
    in_specs = (PartitionSpec("core"),) * (n_params + n_outs)
    out_specs = (PartitionSpec("core"),) * n_outs
    sharded = jax.jit(
        shard_map(_body, mesh=mesh, in_specs=in_specs, out_specs=out_specs,
                  check_rep=False),
        donate_argnums=donate, keep_unused=True)
    shd = NamedSharding(mesh, PartitionSpec("core"))
    concat_in = [
        jax.device_put(
            np.concatenate([np.asarray(in_maps[c][nm])
                            for c in range(N_CORES)], axis=0), shd)
        for nm in in_names[:n_params]
    ]
    def zero_batch():
        return [jax.device_put(
            np.zeros((N_CORES * z.shape[0], *z.shape[1:]), z.dtype), shd)
            for z in zero_outs]

    out = sharded(*concat_in, *zero_batch())
    jax.block_until_ready(out)
    times = []
    for _ in range(iters):
        zb = zero_batch()
        jax.block_until_ready(zb)
        t0 = time.perf_counter()
        out = sharded(*concat_in, *zb)
        jax.block_until_ready(out)
        times.append(time.perf_counter() - t0)
    return min(times) * 1e9


def make_runner(nc, inputs):
    """Build a callable that dispatches nc once (device-resident inputs)
    and returns wall seconds. For differential timing."""
    import time
    import jax
    from jax.sharding import Mesh, PartitionSpec, NamedSharding
    from jax.experimental.shard_map import shard_map
    from concourse import bass2jax, mybir

    in_maps = _prep(inputs)
    bass2jax.install_neuronx_cc_hook()
    pn = nc.partition_id_tensor.name if nc.partition_id_tensor else None
    in_names, out_names, out_avals, zero_outs = [], [], [], []
    for alloc in nc.m.functions[0].allocations:
        if not isinstance(alloc, mybir.MemoryLocationSet):
            continue
        name = alloc.memorylocations[0].name
        if alloc.kind == "ExternalInput":
            if name != pn:
                in_names.append(name)
        elif alloc.kind == "ExternalOutput":
            shape = tuple(alloc.tensor_shape)
            dtype = mybir.dt.np(alloc.dtype)
            out_names.append(name)
            out_avals.append(jax.core.ShapedArray(shape, dtype))
            zero_outs.append(np.zeros(shape, dtype))
    n_params, n_outs = len(in_names), len(out_avals)
    in_names = in_names + out_names
    if pn is not None:
        in_names.append(pn)
    donate = tuple(range(n_params, n_params + n_outs))

    def _body(*args):
        operands = list(args)
        if pn is not None:
            operands.append(bass2jax.partition_id_tensor())
        return tuple(bass2jax._bass_exec_p.bind(
            *operands, out_avals=tuple(out_avals), in_names=tuple(in_names),
            out_names=tuple(out_names), lowering_input_output_aliases=(),
            sim_require_finite=True, sim_require_nnan=True, nc=nc))

    mesh = Mesh(np.asarray(jax.devices()[:N_CORES]), ("core",))
    sharded = jax.jit(
        shard_map(_body, mesh=mesh,
                  in_specs=(PartitionSpec("core"),) * (n_params + n_outs),
                  out_specs=(PartitionSpec("core"),) * n_outs,
                  check_rep=False),
        donate_argnums=donate, keep_unused=True)
    shd = NamedSharding(mesh, PartitionSpec("core"))
    concat_in = [
        jax.device_put(
            np.concatenate([np.asarray(in_maps[c][nm])
                            for c in range(N_CORES)], axis=0), shd)
        for nm in in_names[:n_params]
    ]

    def run_once():
        zb = [jax.device_put(
            np.zeros((N_CORES * z.shape[0], *z.shape[1:]), z.dtype), shd)
            for z in zero_outs]
        jax.block_until_ready(zb)
        t0 = time.perf_counter()
        out = sharded(*concat_in, *zb)
        jax.block_until_ready(out)
        return time.perf_counter() - t0

    run_once()  # warmup/compile
    return run_once

